# revision 5
# baseline (speedup 1.0000x reference)
# Sparse-attention kernel for Trainium2 (8 NeuronCores, SPMD).
#
# Sharding: core c handles batch b=c//4, query rows i0=(c%4)*512 .. +512, all
# 8 heads. No collectives: each core produces its own output rows; the host
# concatenates. The 256MB attn_bias tensor (the memory-roofline term) is
# host-pre-transposed per core to [h, j, i] so all on-chip work runs in
# transposed-score layout [j(part), i(free)]:
#   sT = k @ qT (PE, K=32) ; psum += bias via identity-matmul ; p = exp(psum) (ACT)
#   outT[c,i] = sum_j Vaug[j,c] * p[j,i]  (PE, K=128 accumulate over j-tiles)
# Vaug = [v*mask_j ; mask_j] folds column masking and the softmax denominator
# into the PV matmul. Fully-masked query rows (reference: uniform attention =
# mean of v) are fixed with a per-column blend against host-computed vmean.
import numpy as np

import concourse.bass as bass
import concourse.bacc as bacc
import concourse.mybir as mybir
import concourse.tile as tile
from concourse.bass_utils import run_bass_kernel_spmd
from concourse.masks import make_identity

B, S, D, H, DH = 2, 2048, 256, 8, 32
INNER = H * DH          # 256
NCORES = 8
SI = 512                # query rows per core
JT = S // 128           # 16 j-tiles
SCALE = DH ** -0.5
F32 = mybir.dt.float32

_BUILD_CACHE = {}


def build_bass():
    if "nc" in _BUILD_CACHE:
        return _BUILD_CACHE["nc"]
    nc = bacc.Bacc()
    xt = nc.declare_dram_parameter("xt", [D, S], F32, isOutput=False)
    xq = nc.declare_dram_parameter("xq", [D, SI], F32, isOutput=False)
    bias_t = nc.declare_dram_parameter("bias_t", [H, S, SI], F32, isOutput=False)
    maskp = nc.declare_dram_parameter("maskp", [128, JT], F32, isOutput=False)
    maski = nc.declare_dram_parameter("maski", [1, SI], F32, isOutput=False)
    wq = nc.declare_dram_parameter("wq", [D, INNER], F32, isOutput=False)
    wk = nc.declare_dram_parameter("wk", [D, INNER], F32, isOutput=False)
    wv = nc.declare_dram_parameter("wv", [D, INNER], F32, isOutput=False)
    wg = nc.declare_dram_parameter("wg", [D, INNER], F32, isOutput=False)
    wo = nc.declare_dram_parameter("wo", [INNER, D], F32, isOutput=False)
    bgc = nc.declare_dram_parameter("bgc", [128, 2], F32, isOutput=False)
    boc = nc.declare_dram_parameter("boc", [128, 2], F32, isOutput=False)
    vmr = nc.declare_dram_parameter("vmr", [1, INNER], F32, isOutput=False)
    out_t = nc.declare_dram_parameter("out_t", [D, SI], F32, isOutput=True)

    AF = mybir.ActivationFunctionType

    with tile.TileContext(nc) as tc:
        with (
            tc.tile_pool(name="consts", bufs=1) as consts,
            tc.tile_pool(name="ps", bufs=2, space="PSUM") as psp,
            tc.tile_pool(name="pvp", bufs=2, space="PSUM") as pvp,
            tc.tile_pool(name="epp", bufs=2, space="PSUM") as epp,
            tc.tile_pool(name="biasp", bufs=3) as biasp,
            tc.tile_pool(name="ptp", bufs=8) as ptp,
            tc.tile_pool(name="rowp", bufs=4) as rowp,
        ):
            # ---- constants / inputs to SBUF ----
            xt_sb = consts.tile([128, 2, S], F32)
            nc.sync.dma_start(xt_sb, xt[:].rearrange("(c p) j -> p c j", p=128))
            xq_sb = consts.tile([128, 2, SI], F32)
            nc.sync.dma_start(xq_sb, xq[:].rearrange("(c p) i -> p c i", p=128))
            w_sb = {}
            for name, wh in (("wq", wq), ("wk", wk), ("wv", wv), ("wg", wg), ("wo", wo)):
                t = consts.tile([128, 2, INNER], F32, tag=f"w_{name}")
                nc.sync.dma_start(t, wh[:].rearrange("(c p) o -> p c o", p=128))
                w_sb[name] = t
            maskp_sb = consts.tile([128, JT], F32)
            nc.sync.dma_start(maskp_sb, maskp[:])
            maski_sb = consts.tile([1, SI], F32)
            nc.sync.dma_start(maski_sb, maski[:])
            bgc_sb = consts.tile([128, 2], F32)
            nc.sync.dma_start(bgc_sb, bgc[:])
            boc_sb = consts.tile([128, 2], F32)
            nc.sync.dma_start(boc_sb, boc[:])
            vmr_sb = consts.tile([1, INNER], F32)
            nc.sync.dma_start(vmr_sb, vmr[:])

            ident = consts.tile([128, 128], F32)
            make_identity(nc, ident)
            ones8 = consts.tile([128, 8], F32)
            nc.gpsimd.memset(ones8, 1.0)
            ones32 = consts.tile([1, 32], F32)
            nc.gpsimd.memset(ones32, 1.0)
            onemi = consts.tile([1, SI], F32)
            nc.vector.tensor_scalar(onemi, maski_sb, -1.0, 1.0,
                                    mybir.AluOpType.mult, mybir.AluOpType.add)

            # ---- projections ----
            # qT[o*128+p, i] (scaled), kT[o*128+p, j]
            qt_sb = consts.tile([128, 2, SI], F32)
            for o in range(2):
                p = psp.tile([128, 1024], F32, tag="ps")
                for kc in range(2):
                    nc.tensor.matmul(p[:, :SI], w_sb["wq"][:, kc, o * 128:(o + 1) * 128],
                                     xq_sb[:, kc, :], start=(kc == 0), stop=(kc == 1))
                nc.vector.tensor_scalar_mul(qt_sb[:, o], p[:, :SI], SCALE)
            kt_sb = consts.tile([128, 2, S], F32)
            for o in range(2):
                for nj in range(4):
                    p = psp.tile([128, 1024], F32, tag="ps")
                    for kc in range(2):
                        nc.tensor.matmul(p[:, :512], w_sb["wk"][:, kc, o * 128:(o + 1) * 128],
                                         xt_sb[:, kc, nj * 512:(nj + 1) * 512],
                                         start=(kc == 0), stop=(kc == 1))
                    nc.vector.tensor_copy(kt_sb[:, o, nj * 512:(nj + 1) * 512], p[:, :512])

            # v in [j, inner] layout, masked, interleaved with mask column:
            # vaug[p, jt, h*33 .. h*33+31] = v[jt*128+p, h*32..] * mask_j
            # vaug[p, jt, h*33+32]         = mask_j
            vaug_sb = consts.tile([128, JT, 33 * H], F32)
            for jt in range(JT):
                p = psp.tile([128, 1024], F32, tag="ps")
                for kc in range(2):
                    nc.tensor.matmul(p[:, :INNER], xt_sb[:, kc, jt * 128:(jt + 1) * 128],
                                     w_sb["wv"][:, kc, :], start=(kc == 0), stop=(kc == 1))
                va = vaug_sb[:, jt].rearrange("p (h c) -> p h c", c=33)
                nc.vector.tensor_scalar_mul(
                    va[:, :, :32], p[:, :INNER].rearrange("p (h c) -> p h c", c=32),
                    maskp_sb[:, jt:jt + 1])
                nc.vector.tensor_scalar_mul(va[:, :, 32:33], ones8[:, :, None],
                                            maskp_sb[:, jt:jt + 1])

            # gates: gT = sigmoid(Wg^T xq + bg)
            gt_sb = consts.tile([128, 2, SI], F32)
            for o in range(2):
                p = psp.tile([128, 1024], F32, tag="ps")
                for kc in range(2):
                    nc.tensor.matmul(p[:, :SI], w_sb["wg"][:, kc, o * 128:(o + 1) * 128],
                                     xq_sb[:, kc, :], start=(kc == 0), stop=(kc == 1))
                nc.scalar.activation(gt_sb[:, o], p[:, :SI], AF.Sigmoid,
                                     bias=bgc_sb[:, o:o + 1], scale=1.0)

            # ---- attention, head by head ----
            ot_sb = consts.tile([128, 2, SI], F32)
            for h in range(H):
                hb, hp = divmod(h, 4)
                po = pvp.tile([33, SI], F32, tag="pv")
                for jg in range(8):  # two j-tiles per group
                    bt = biasp.tile([128, 1024], F32, tag="bias")
                    nc.sync.dma_start(
                        bt.rearrange("p (t i) -> p t i", t=2),
                        bias_t[h, jg * 256:(jg + 1) * 256, :].rearrange(
                            "(t p) i -> p t i", p=128))
                    p = psp.tile([128, 1024], F32, tag="ps")
                    for t in range(2):
                        nc.tensor.matmul(p[:, t * 512:(t + 1) * 512], ident,
                                         bt[:, t * 512:(t + 1) * 512],
                                         start=True, stop=False, skip_group_check=True)
                    for t in range(2):
                        jt = jg * 2 + t
                        nc.tensor.matmul(
                            p[:, t * 512:(t + 1) * 512],
                            kt_sb[hp * 32:(hp + 1) * 32, hb, jt * 128:(jt + 1) * 128],
                            qt_sb[hp * 32:(hp + 1) * 32, hb, :],
                            start=False, stop=True, skip_group_check=True,
                            tile_position=(hp * 32, 0))
                    pt = ptp.tile([128, 1024], F32, tag="pt")
                    nc.scalar.activation(pt, p, AF.Exp)
                    for t in range(2):
                        jt = jg * 2 + t
                        nc.tensor.matmul(po, vaug_sb[:, jt, h * 33:(h + 1) * 33],
                                         pt[:, t * 512:(t + 1) * 512],
                                         start=(jt == 0), stop=(jt == JT - 1),
                                         skip_group_check=True)
                # epilogue: normalize + blend invalid rows to vmean
                rinv = rowp.tile([1, SI], F32, tag="row")
                nc.vector.reciprocal(rinv, po[32:33, :])
                rmi = rowp.tile([1, SI], F32, tag="row")
                nc.vector.tensor_mul(rmi, rinv, maski_sb)
                pr = epp.tile([32, SI], F32, tag="ep")
                nc.tensor.matmul(pr, ones32, rmi, start=True, stop=True,
                                 skip_group_check=True)
                pw = epp.tile([32, SI], F32, tag="ep")
                nc.tensor.matmul(pw, vmr_sb[:, h * 32:(h + 1) * 32], onemi,
                                 start=True, stop=True, skip_group_check=True)
                prs = rowp.tile([32, SI], F32, tag="eps")
                nc.vector.tensor_copy(prs, pr)
                dst = ot_sb[hp * 32:(hp + 1) * 32, hb, :]
                nc.vector.tensor_mul(dst, po[0:32, :], prs)
                nc.vector.tensor_add(dst, dst, pw)

            # ---- gating + output projection ----
            og_sb = consts.tile([128, 2, SI], F32)
            nc.vector.tensor_mul(og_sb[:], ot_sb[:], gt_sb[:])
            yt_sb = consts.tile([128, 2, SI], F32)
            for o in range(2):
                p = psp.tile([128, 1024], F32, tag="ps")
                for kc in range(2):
                    nc.tensor.matmul(p[:, :SI], w_sb["wo"][:, kc, o * 128:(o + 1) * 128],
                                     og_sb[:, kc, :], start=(kc == 0), stop=(kc == 1))
                nc.scalar.add(yt_sb[:, o], p[:, :SI], boc_sb[:, o:o + 1])
            nc.sync.dma_start(out_t[:].rearrange("(c p) i -> p c i", p=128), yt_sb)

    nc.finalize()
    _BUILD_CACHE["nc"] = nc
    return nc


def make_in_maps(x, mask, attn_bias, Wq, Wkv, Wo, bo, Wg, bg):
    x = np.ascontiguousarray(x, dtype=np.float32)
    attn_bias = np.asarray(attn_bias, dtype=np.float32)
    Wq = np.ascontiguousarray(Wq, dtype=np.float32)
    Wkv = np.asarray(Wkv, dtype=np.float32)
    Wo = np.ascontiguousarray(Wo, dtype=np.float32)
    Wg = np.ascontiguousarray(Wg, dtype=np.float32)
    bo = np.asarray(bo, dtype=np.float32)
    bg = np.asarray(bg, dtype=np.float32)
    Wk = np.ascontiguousarray(Wkv[:, :INNER])
    Wv = np.ascontiguousarray(Wkv[:, INNER:])
    mf = np.asarray(mask).astype(np.float32)
    bgc = np.ascontiguousarray(bg.reshape(2, 128).T)
    boc = np.ascontiguousarray(bo.reshape(2, 128).T)
    in_maps = []
    for b in range(B):
        xtb = np.ascontiguousarray(x[b].T)               # [D, S]
        v = x[b] @ Wv                                    # [S, INNER]
        vmrb = np.ascontiguousarray(v.mean(axis=0, dtype=np.float32).reshape(1, INNER))
        maskpb = np.ascontiguousarray(mf[b].reshape(JT, 128).T)
        for s4 in range(4):
            i0 = s4 * SI
            in_maps.append(dict(
                xt=xtb,
                xq=np.ascontiguousarray(xtb[:, i0:i0 + SI]),
                bias_t=np.ascontiguousarray(
                    attn_bias[b, :, i0:i0 + SI, :].transpose(0, 2, 1)),
                maskp=maskpb,
                maski=np.ascontiguousarray(mf[b, i0:i0 + SI].reshape(1, SI)),
                wq=Wq, wk=Wk, wv=Wv, wg=Wg, wo=Wo,
                bgc=bgc, boc=boc, vmr=vmrb,
            ))
    return in_maps


def run(x, mask, attn_bias, Wq, Wkv, Wo, bo, Wg, bg, **rb_kwargs):
    nc = build_bass()
    in_maps = make_in_maps(x, mask, attn_bias, Wq, Wkv, Wo, bo, Wg, bg)
    res = run_bass_kernel_spmd(nc, in_maps, core_ids=list(range(NCORES)), **rb_kwargs)
    out = np.empty((B, S, D), dtype=np.float32)
    for c in range(NCORES):
        b, s4 = divmod(c, 4)
        out[b, s4 * SI:(s4 + 1) * SI, :] = res.results[c]["out_t"].T
    return out, res


def kernel(x, mask, attn_bias, Wq, Wkv, Wo, bo, Wg, bg):
    out, _ = run(x, mask, attn_bias, Wq, Wkv, Wo, bo, Wg, bg)
    return out
